# revision 4
# baseline (speedup 1.0000x reference)
"""DeltaOnlyModel Trainium2 kernel.

Pure data parallel over batch: 256 examples -> 8 cores x 32.
The per-token encoder collapses to 64-entry token tables (vocab=64, no
position mixing), computed on device. The gated delta-rule scan runs in
chunks of C=128 steps: per-chunk per-example K/theta sequences come from
one-hot matmuls on the tensor engine (quadrant packed), the sequential
gate recurrence runs on the vector engine in [32 ex x 64 H] layout with
fused scalar_tensor_tensor ops, and cross-sub-chunk corrections plus
fast-weight (M) updates are per-example matmuls accumulating in PSUM.
"""

import os
import numpy as np

H = 64
VOC = 64
L = 2048
B = 256
NCORE = 8
BPC = B // NCORE          # 32 examples per core
C = 128                   # chunk length (steps)
SC = 8                    # sub-chunk length (solve window)
NSUB = C // SC
THR2 = 0.4 * 0.4
LN_EPS = 1e-5
NORM_EPS = 1e-12

# test hook: truncate the scan to fewer chunks (kernel then models a
# shorter sequence whose readout token is x[:, NCH*C-1])
NCH = int(os.environ.get("KERNEL_NCH", L // C))
LEFF = NCH * C


def _build(nc, tc, ctx):
    from concourse import mybir
    from concourse.bass import ds
    f32 = mybir.dt.float32
    AL = mybir.AluOpType
    AF = mybir.ActivationFunctionType
    HB = BPC // 2   # examples per partition-half

    def inp(name, shape):
        return nc.dram_tensor(name, shape, f32, kind="ExternalInput").ap()

    xf = nc.dram_tensor("xf", [BPC, L], mybir.dt.uint8,
                        kind="ExternalInput").ap()
    embed = inp("embed", [VOC, H])
    w1 = inp("w1", [H, 2 * H])
    b1 = inp("b1", [2 * H, 1])
    w2 = inp("w2", [2 * H, H])
    b2 = inp("b2", [H, 1])
    ln_g = inp("ln_g", [1, H])
    ln_b = inp("ln_b", [1, H])
    wk = inp("wk", [H, H])
    wv = inp("wv", [H, H])
    wq = inp("wq", [H, H])
    wo = inp("wo", [H, H])
    bo = inp("bo", [H, 1])
    xl = inp("xl", [1, BPC])              # tokens at step LEFF-1
    out_d = nc.dram_tensor("out", [H, BPC], f32, kind="ExternalOutput").ap()

    tabs_d = nc.dram_tensor("tabs_d", [VOC, 3 * H + 1], f32).ap()
    vt_d = nc.dram_tensor("vt_d", [VOC, H], f32).ap()
    aneg_d = nc.dram_tensor("aneg_d", [2, BPC, C, C], f32).ap()
    u_d = nc.dram_tensor("u_d", [2, C, BPC, H], f32).ap()
    rfull_d = nc.dram_tensor("rfull_d", [C, BPC, H], f32).ap()
    acd_d = nc.dram_tensor("acd_d", [BPC, NSUB * SC, SC], f32).ap()
    th_d = nc.dram_tensor("th_d", [BPC, C], f32).ap()

    cst = ctx.enter_context(tc.tile_pool(name="cst", bufs=1))
    tabp = ctx.enter_context(tc.tile_pool(name="tabp", bufs=1))
    big = ctx.enter_context(tc.tile_pool(name="big", bufs=1))
    dbl = ctx.enter_context(tc.tile_pool(name="dbl", bufs=2))
    sol = ctx.enter_context(tc.tile_pool(name="sol", bufs=3))
    smp = ctx.enter_context(tc.tile_pool(name="smp", bufs=3))
    stp = ctx.enter_context(tc.tile_pool(name="stp", bufs=2))
    rrp = ctx.enter_context(tc.tile_pool(name="rrp", bufs=2))
    pst = ctx.enter_context(tc.tile_pool(name="pst", bufs=4, space="PSUM"))
    psr = ctx.enter_context(tc.tile_pool(name="psr", bufs=1, space="PSUM"))

    # ============ phase 0: token tables ============
    embT = cst.tile([H, VOC], f32)
    nc.sync.dma_start(embT[:], embed.rearrange("a b -> b a"))
    w1s = cst.tile([H, 2 * H], f32)
    nc.sync.dma_start(w1s[:], w1)
    b1s = cst.tile([2 * H, 1], f32)
    nc.sync.dma_start(b1s[:], b1)
    w2s = cst.tile([2 * H, H], f32)
    nc.sync.dma_start(w2s[:], w2)
    b2s = cst.tile([H, 1], f32)
    nc.sync.dma_start(b2s[:], b2)
    gRow = cst.tile([VOC, H], f32)
    nc.sync.dma_start(gRow[:], ln_g.broadcast_to([VOC, H]))
    bRow = cst.tile([VOC, H], f32)
    nc.sync.dma_start(bRow[:], ln_b.broadcast_to([VOC, H]))
    wks = cst.tile([H, H], f32)
    nc.sync.dma_start(wks[:], wk)
    wvs = cst.tile([H, H], f32)
    nc.sync.dma_start(wvs[:], wv)
    wqs = cst.tile([H, H], f32)
    nc.sync.dma_start(wqs[:], wq)
    wos = cst.tile([H, H], f32)
    nc.sync.dma_start(wos[:], wo)
    bos = cst.tile([H, 1], f32)
    nc.sync.dma_start(bos[:], bo)
    # iot[p] = p % 64 and idn = I_128, generated on device
    ioti = cst.tile([128, 1], mybir.dt.int32)
    nc.gpsimd.iota(ioti[:], pattern=[[0, 1]], base=0, channel_multiplier=1)
    iot = cst.tile([128, 1], f32)
    nc.scalar.copy(iot[:], ioti[:])
    hi = cst.tile([128, 1], f32)
    nc.vector.tensor_scalar(hi[:], iot[:], float(VOC), None, op0=AL.is_ge)
    nc.vector.scalar_tensor_tensor(iot[:], hi[:], -float(VOC), iot[:],
                                   op0=AL.mult, op1=AL.add)
    idn = cst.tile([128, 128], f32)
    nc.gpsimd.memset(idn[:], 0.0)
    nc.gpsimd.affine_select(out=idn[:], in_=idn[:],
                            compare_op=AL.not_equal, fill=1.0, base=0,
                            pattern=[[-1, 128]], channel_multiplier=1)

    ps1 = pst.tile([2 * H, VOC], f32, tag="ps")
    nc.tensor.matmul(ps1[:], w1s[:], embT[:], start=True, stop=True)
    r1 = smp.tile([2 * H, VOC], f32)
    nc.scalar.activation(r1[:], ps1[:], AF.Relu, bias=b1s[:], scale=1.0)
    ps2 = pst.tile([H, VOC], f32, tag="ps")
    nc.tensor.matmul(ps2[:], w2s[:], r1[:], start=True, stop=True)
    hpreT = smp.tile([H, VOC], f32)
    nc.vector.scalar_tensor_tensor(hpreT[:], ps2[:], b2s[:], embT[:],
                                   op0=AL.add, op1=AL.add)
    ps3 = pst.tile([VOC, H], f32, tag="ps")
    nc.tensor.transpose(ps3[:], hpreT[:], idn[0:H, 0:VOC])
    hp = smp.tile([VOC, H], f32)
    nc.scalar.copy(hp[:], ps3[:])
    mu = smp.tile([VOC, 1], f32)
    nc.vector.reduce_sum(mu[:], hp[:], axis=mybir.AxisListType.X)
    nc.vector.tensor_scalar(mu[:], mu[:], 1.0 / H, None, op0=AL.mult)
    xc = smp.tile([VOC, H], f32)
    nc.vector.tensor_scalar(xc[:], hp[:], mu[:], None, op0=AL.subtract)
    var = smp.tile([VOC, 1], f32)
    sq = smp.tile([VOC, H], f32)
    nc.vector.scalar_tensor_tensor(sq[:], xc[:], 1.0, xc[:],
                                   op0=AL.bypass, op1=AL.mult, accum_out=var[:])
    rstd = smp.tile([VOC, 1], f32)
    nc.vector.tensor_scalar(rstd[:], var[:], 1.0 / H, LN_EPS,
                            op0=AL.mult, op1=AL.add)
    nc.scalar.activation(rstd[:], rstd[:], AF.Sqrt)
    nc.vector.reciprocal(rstd[:], rstd[:])
    hn = smp.tile([VOC, H], f32)
    nc.vector.tensor_scalar(hn[:], xc[:], rstd[:], None, op0=AL.mult)
    nc.vector.tensor_mul(hn[:], hn[:], gRow[:])
    nc.vector.tensor_add(hn[:], hn[:], bRow[:])
    ps4 = pst.tile([H, VOC], f32, tag="ps")
    nc.tensor.transpose(ps4[:], hn[:], idn[0:VOC, 0:H])
    hnT = smp.tile([H, VOC], f32)
    nc.scalar.copy(hnT[:], ps4[:])

    psk = pst.tile([VOC, 3 * H], f32, tag="ps")
    nc.tensor.matmul(psk[:, 0:H], hnT[:], wks[:], start=True, stop=True)
    nc.tensor.matmul(psk[:, H:2 * H], hnT[:], wvs[:], start=True, stop=True)
    nc.tensor.matmul(psk[:, 2 * H:3 * H], hnT[:], wqs[:], start=True, stop=True)
    kvq = smp.tile([VOC, 3 * H], f32)
    nc.vector.tensor_copy(kvq[:], psk[:])
    kn2 = smp.tile([VOC, 1], f32)
    ksq = smp.tile([VOC, H], f32)
    nc.vector.scalar_tensor_tensor(ksq[:], kvq[:, 0:H], 1.0, kvq[:, 0:H],
                                   op0=AL.bypass, op1=AL.mult, accum_out=kn2[:])
    rkn = smp.tile([VOC, 1], f32)
    nc.scalar.activation(rkn[:], kn2[:], AF.Sqrt)
    nc.vector.tensor_scalar(rkn[:], rkn[:], NORM_EPS, None, op0=AL.max)
    nc.vector.reciprocal(rkn[:], rkn[:])
    rknn = smp.tile([VOC, 1], f32)
    nc.vector.tensor_scalar(rknn[:], rkn[:], -1.0, None, op0=AL.mult)

    tabs = smp.tile([VOC, 3 * H + 1], f32)
    nc.vector.tensor_scalar(tabs[:, 0:H], kvq[:, 0:H], rkn[:], None, op0=AL.mult)
    nc.vector.tensor_scalar(tabs[:, H:2 * H], kvq[:, 0:H], rknn[:], None,
                            op0=AL.mult)
    nc.vector.tensor_scalar(tabs[:, 2 * H:3 * H], kvq[:, 2 * H:3 * H], -1.0,
                            None, op0=AL.mult)
    vtab = smp.tile([VOC, H], f32)
    nc.vector.tensor_copy(vtab[:], kvq[:, H:2 * H])
    th1 = smp.tile([VOC, 1], f32)
    vsq = smp.tile([VOC, H], f32)
    nc.vector.scalar_tensor_tensor(vsq[:], vtab[:], 1.0, vtab[:],
                                   op0=AL.bypass, op1=AL.mult, accum_out=th1[:])
    nc.vector.tensor_scalar(tabs[:, 3 * H:3 * H + 1], th1[:], THR2, None,
                            op0=AL.mult)

    nc.sync.dma_start(tabs_d, tabs[:])
    nc.sync.dma_start(vt_d, vtab[:])
    tab2 = tabp.tile([128, 3 * H + 1], f32)
    nc.sync.dma_start(tab2[0:VOC, :], tabs_d)
    nc.sync.dma_start(tab2[VOC:128, :], tabs_d)
    vtab2 = tabp.tile([128, H], f32)
    nc.sync.dma_start(vtab2[0:VOC, :], vt_d)
    nc.sync.dma_start(vtab2[VOC:128, :], vt_d)

    Mneg = tabp.tile([H, BPC * H], f32)   # -M^T per example
    nc.vector.memzero(Mneg[:])

    # ============ phase 1: chunks (hardware loop) ============
    # Single chunk body traced once; DRAM scratch single-buffered (the
    # loop's iteration barrier serializes cross-iteration hazards).
    with tc.For_i(0, LEFF, C, name="chunk") as cs:
        db = 0
        xb8 = big.tile([128, HB, C], mybir.dt.uint8, tag="xb8")
        nc.sync.dma_start(xb8[0:VOC, :, :],
                          xf[0:HB, ds(cs, C)].unsqueeze(0)
                          .broadcast_to([VOC, HB, C]))
        nc.sync.dma_start(xb8[VOC:128, :, :],
                          xf[HB:BPC, ds(cs, C)].unsqueeze(0)
                          .broadcast_to([VOC, HB, C]))
        xb = big.tile([128, HB, C], f32, tag="xb")
        nc.scalar.copy(xb[:], xb8[:])
        oh = big.tile([128, HB, C], f32, tag="oh")
        nc.vector.tensor_scalar(oh[:], xb[:], iot[:], None, op0=AL.is_equal)

        kall = dbl.tile([C, BPC, 2 * H], f32, tag="kall")
        thcol_all = smp.tile([C, BPC], f32, tag="thcol")
        ktall = dbl.tile([H, BPC, C], f32, tag="ktall")
        ktnall = big.tile([H, BPC, C], f32, tag="ktnall")
        for g in range(BPC // 4):
            psa = pst.tile([C, 4, 2 * H], f32, tag="ps")
            psth = pst.tile([C, 4, 1], f32, tag="ps")
            psbT = pst.tile([H, 4, C], f32, tag="ps")
            psbTn = pst.tile([H, 4, C], f32, tag="ps")
            for j in range(4):
                e = g * 4 + j
                half = 0 if e < HB else VOC
                es = e if e < HB else e - HB
                ohs = oh[half:half + VOC, es, :]
                nc.tensor.matmul(psa[:, j, :], ohs,
                                 tab2[half:half + VOC, 0:2 * H],
                                 start=True, stop=True, tile_position=(half, 0))
                nc.tensor.matmul(psth[:, j, :], ohs,
                                 tab2[half:half + VOC, 3 * H:3 * H + 1],
                                 start=True, stop=True, tile_position=(half, 0))
                nc.tensor.matmul(psbT[:, j, :],
                                 tab2[half:half + VOC, 0:H], ohs,
                                 start=True, stop=True, tile_position=(half, 0))
                nc.tensor.matmul(psbTn[:, j, :],
                                 tab2[half:half + VOC, H:2 * H], ohs,
                                 start=True, stop=True, tile_position=(half, 0))
            nc.scalar.copy(kall[:, g * 4:(g + 1) * 4, :], psa[:])
            nc.scalar.copy(thcol_all[:, g * 4:(g + 1) * 4], psth[:, :, 0])
            nc.scalar.copy(ktall[:, g * 4:(g + 1) * 4, :], psbT[:])
            nc.scalar.copy(ktnall[:, g * 4:(g + 1) * 4, :], psbTn[:])

        thps = pst.tile([BPC, C], f32, tag="ps")
        nc.tensor.transpose(thps[:], thcol_all[:], idn[0:C, 0:C])
        thb = sol.tile([BPC, C], f32, tag="thb")
        nc.scalar.copy(thb[:], thps[:])

        for g in range(BPC // 4):
            pan = pst.tile([C, 4, C], f32, tag="ps")
            ansb = smp.tile([C, 4, C], f32, tag="ansb")
            for j in range(4):
                e = g * 4 + j
                nc.tensor.matmul(pan[:, j, :], ktall[:, e, :], ktnall[:, e, :],
                                 start=True, stop=True)
            nc.scalar.copy(ansb[:], pan[:])
            nc.sync.dma_start(aneg_d[db, g * 4:(g + 1) * 4].transpose([1, 0, 2]),
                              ansb[:])
        acols = sol.tile([BPC, NSUB * SC, SC], f32, tag="acols")
        for J in range(NSUB):
            nc.sync.dma_start(
                acols[:, J * SC:(J + 1) * SC, :],
                aneg_d[db, :, J * SC:(J + 1) * SC, J * SC:(J + 1) * SC])
        nc.sync.dma_start(acd_d, acols[:])
        nc.sync.dma_start(th_d, thb[:])

        # r'' init: V gather, then -K M^T
        # PSUM group discipline: one start=True per bank (8 examples/bank)
        # per chunk; everything else accumulates via per-element has_written.
        rps = psr.tile([C, BPC, H], f32, tag="rps")
        for e in range(BPC):
            half = 0 if e < HB else VOC
            es = e if e < HB else e - HB
            ohs = oh[half:half + VOC, es, :]
            nc.tensor.matmul(rps[:, e, :], ohs, vtab2[half:half + VOC, :],
                             start=(e % 8 == 0), stop=False,
                             tile_position=(half, 0), skip_group_check=True)
        laststop = NSUB < 2
        for e in range(BPC):
            nc.tensor.matmul(rps[:, e, :], ktall[:, e, :],
                             Mneg[:, e * H:(e + 1) * H],
                             start=False, stop=(laststop and e % 8 == 7),
                             skip_group_check=True)

        # sub-chunk solve as an inner hardware loop: all J-dependent
        # accesses go through DRAM (dynamic DMA offsets); compute-engine
        # APs stay static. Corrections write full rows (past rows are
        # dead) so the matmul shape is loop-invariant.
        uc = big.tile([C, BPC, H], f32, tag="uc")
        with tc.For_i(0, C, SC, name="sub") as jj:
            rrow = rrp.tile([C, BPC, H], f32, tag="rrow")
            nc.scalar.copy(rrow[:], rps[:])
            nc.sync.dma_start(rfull_d, rrow[:])
            rb = sol.tile([BPC, SC, H], f32, tag="rb")
            nc.sync.dma_start(rb[:],
                              rfull_d[ds(jj, SC), :, :].transpose([1, 0, 2]))
            acJ = sol.tile([BPC, SC, SC], f32, tag="acJ")
            nc.sync.dma_start(acJ[:], acd_d[:, ds(jj, SC), :])
            thJ = sol.tile([BPC, SC], f32, tag="thJ")
            nc.sync.dma_start(thJ[:], th_d[:, ds(jj, SC)])

            for k in range(SC):
                dslot = rb[:, k, :]
                for s in range(k):
                    nc.vector.scalar_tensor_tensor(
                        dslot, rb[:, s, :], acJ[:, s, k:k + 1], dslot,
                        op0=AL.mult, op1=AL.add)
                nsc = smp.tile([BPC, H], f32, tag="nsc")
                ncol = smp.tile([BPC, 1], f32, tag="ncol")
                nc.vector.scalar_tensor_tensor(nsc[:], dslot, 1.0, dslot,
                                               op0=AL.bypass, op1=AL.mult,
                                               accum_out=ncol[:])
                nc.vector.scalar_tensor_tensor(
                    dslot, ncol[:].broadcast_to([BPC, H]), thJ[:, k:k + 1],
                    dslot, op0=AL.is_gt, op1=AL.mult)
            nc.sync.dma_start(
                u_d[db, ds(jj, SC), :, :].transpose([1, 0, 2]), rb[:])

            # 4-way packed strip corrections at partition rows 0/32/64/96
            usub = stp.tile([128, BPC // 4, H], f32, tag="usub")
            strip = stp.tile([128, BPC // 4, C], f32, tag="strip")
            for q in range(4):
                es, ee = q * 8, (q + 1) * 8
                nc.sync.dma_start(usub[32 * q:32 * q + SC, :, :],
                                  u_d[db, ds(jj, SC), es:ee, :])
                nc.sync.dma_start(
                    strip[32 * q:32 * q + SC, :, :],
                    aneg_d[db, es:ee, ds(jj, SC), :].transpose([1, 0, 2]))
            for e in range(BPC):
                q, er = e // 8, e % 8
                nc.tensor.matmul(rps[:, e, :],
                                 strip[32 * q:32 * q + SC, er, :],
                                 usub[32 * q:32 * q + SC, er, :],
                                 start=False, stop=False,
                                 skip_group_check=True,
                                 tile_position=(32 * q, 0))

        nc.sync.dma_start(uc[:], u_d[db])
        dmp = psr.tile([H, BPC, H], f32, tag="rps")
        for e in range(BPC):
            nc.tensor.matmul(dmp[:, e, :], kall[:, e, H:2 * H], uc[:, e, :],
                             start=True, stop=True)
        nc.vector.tensor_add(Mneg[:], Mneg[:],
                             dmp[:].rearrange("j e h -> j (e h)"))

    # ============ phase 2: readout ============
    xlb = smp.tile([VOC, BPC], f32, tag="xlb")
    nc.sync.dma_start(xlb[:], xl.broadcast_to([VOC, BPC]))
    ohl = smp.tile([VOC, BPC], f32, tag="ohl")
    nc.vector.tensor_scalar(ohl[:], xlb[:], iot[0:VOC, :], None,
                            op0=AL.is_equal)
    psq = pst.tile([H, BPC], f32, tag="ps")
    nc.tensor.matmul(psq[:], tab2[0:VOC, 2 * H:3 * H], ohl[:],
                     start=True, stop=True)
    qng = smp.tile([H, BPC], f32, tag="qng")
    nc.scalar.copy(qng[:], psq[:])
    prd = pst.tile([H, BPC], f32, tag="ps")
    for e in range(BPC):
        nc.tensor.matmul(prd[:, e:e + 1], Mneg[:, e * H:(e + 1) * H],
                         qng[:, e:e + 1], start=True, stop=True)
    rd = smp.tile([H, BPC], f32, tag="rd")
    nc.scalar.activation(rd[:], prd[:], AF.Relu)
    pso = pst.tile([H, BPC], f32, tag="ps")
    nc.tensor.matmul(pso[:], wos[:], rd[:], start=True, stop=True)
    ot = smp.tile([H, BPC], f32, tag="ot")
    nc.vector.tensor_scalar(ot[:], pso[:], bos[:], None, op0=AL.add)
    nc.sync.dma_start(out_d, ot[:])


_JAX_CACHE_SET = False


def _setup_jax_cache():
    """Enable jax's persistent compilation cache so repeated
    run_bass_kernel_spmd calls (each of which re-jits a fresh closure)
    hit a deserialized executable instead of recompiling."""
    global _JAX_CACHE_SET
    if _JAX_CACHE_SET:
        return
    try:
        import jax
        os.makedirs("/tmp/.bass_jaxcache", exist_ok=True)
        jax.config.update("jax_compilation_cache_dir", "/tmp/.bass_jaxcache")
        jax.config.update("jax_persistent_cache_min_entry_size_bytes", -1)
        jax.config.update("jax_persistent_cache_min_compile_time_secs", 0.0)
    except Exception:
        pass
    _JAX_CACHE_SET = True


_NC = None


def build_nc():
    global _NC
    if _NC is not None:
        return _NC
    _setup_jax_cache()
    from concourse import bacc
    import concourse.tile as tile
    from contextlib import ExitStack
    nc = bacc.Bacc("TRN2", target_bir_lowering=False, debug=False,
                   num_devices=NCORE)
    with tile.TileContext(nc) as tc:
        with ExitStack() as ctx:
            _build(nc, tc, ctx)
    nc.compile()
    _NC = nc
    return nc


def make_in_maps(inputs):
    x = np.asarray(inputs["x"]).astype(np.int64)
    consts = {
        "embed": inputs["embed"], "w1": inputs["w1"],
        "b1": np.asarray(inputs["b1"]).reshape(2 * H, 1),
        "w2": inputs["w2"], "b2": np.asarray(inputs["b2"]).reshape(H, 1),
        "ln_g": np.asarray(inputs["ln_g"]).reshape(1, H),
        "ln_b": np.asarray(inputs["ln_b"]).reshape(1, H),
        "wk": inputs["wk"], "wv": inputs["wv"], "wq": inputs["wq"],
        "wo": inputs["wo"], "bo": np.asarray(inputs["bo"]).reshape(H, 1),
    }
    consts = {k: np.ascontiguousarray(np.asarray(v, dtype=np.float32))
              for k, v in consts.items()}
    in_maps = []
    for c in range(NCORE):
        m = dict(consts)
        m["xf"] = np.ascontiguousarray(
            x[c * BPC:(c + 1) * BPC].astype(np.uint8))
        m["xl"] = np.ascontiguousarray(
            x[c * BPC:(c + 1) * BPC, LEFF - 1].astype(np.float32).reshape(1, BPC))
        in_maps.append(m)
    return in_maps


_WARMED = False


def kernel(**inputs):
    global _WARMED
    _setup_jax_cache()
    from concourse.bass_utils import run_bass_kernel_spmd
    nc = build_nc()
    in_maps = make_in_maps(inputs)
    res = run_bass_kernel_spmd(nc, in_maps, list(range(NCORE)))
    if not _WARMED:
        # first call in a process pays jit compile / executable load;
        # run once more so a fully-warm execution exists in-process
        _WARMED = True
        res = run_bass_kernel_spmd(nc, in_maps, list(range(NCORE)))
    outs = []
    for c in range(NCORE):
        o = np.asarray(res.results[c]["out"])   # [H, BPC]
        outs.append(o.T)
    return np.concatenate(outs, axis=0).astype(np.float32)

